# revision 71
# baseline (speedup 1.0000x reference)
"""ActionVLAD (vq_codebook) distributed Bass kernel for 8 TRN2 NeuronCores.

Sharding: data-parallel over the 30 timesteps (padded to 32 = 8 cores x 4);
each core runs the full per-timestep VLAD pipeline for its 4 timesteps and
accumulates a partial sum of the globally-normalized VLAD vectors; the host
sums the 8 partial [64, 512] outputs (the all-reduce of the sharding hint).

Numerics strategy:
  - logits W@x computed as a 3-pass bf16 split (Wh@xh + Wh@xl + Wl@xh):
    ~2^-17 relative accuracy at 1 PE-cycle/row (fp32 matmul costs 4).
  - softmax in [n, k] layout after a PE transpose of the logits.
  - VLAD aggregation in fp16 with a 4096x anti-subnormal scale (cancels
    exactly in the intra-normalization); cluster masses in fp32 via PE.
  - all 1/sqrt and 1/x computed as exp(-0.5*ln x) / exp(-ln x) plus one
    Newton step on the vector engine: the whole kernel then uses a single
    activation table (ln/exp/square/copy) -> no ACT table reloads.
"""

import numpy as np

NCORES = 8
TLOC = 4            # local timesteps per core (30 padded to 32)
C = 512
N = 784             # 28*28
K = 64
CCH = 4             # channel chunks of 128
NCH = 7             # spatial chunks
NP = 112            # partitions per spatial chunk
EPS = 1e-12
SCL = 4096.0        # fp16 anti-flush scale on a' (cancels in intra-norm)
C2 = SCL * EPS      # intra-norm clamp in the scaled domain
GEPS2 = 1e-24       # EPS**2 for the global-norm clamp

# engine split knobs (chunk index -> True = ACT, False = DVE)
SUMSQ_ON_ACT = (True, True, False, False, False, False, False)
ZC_ON_ACT = (True, True, True, True, True, True, True)

_CACHE = {}


def _patch_act_tables():
    """Steer the activation-table chooser to natural_log_exp_and_others.

    The kernel only uses ln/exp/square/copy/identity, which all live in that
    one set; by default the greedy chooser alternates natural_log <->
    exp_and_others, inserting a 1.3us table reload around every Ln/Exp pair.
    Emptying the other sets (ids preserved) pins everything to one table.
    """
    import concourse.hw_specs as hw_specs

    if getattr(hw_specs, "_act_tables_patched", False):
        return
    orig = hw_specs.get_activation_tables

    def patched(module_arch):
        tables = dict(orig(module_arch))
        keep = "natural_log_exp_and_others"
        if keep in tables:
            for name in tables:
                if name != keep:
                    tables[name] = set()
        return tables

    patched.cache_clear = lambda: None
    hw_specs.get_activation_tables = patched
    import concourse.bacc as bacc_mod

    bacc_mod.get_activation_tables = patched
    hw_specs._act_tables_patched = True


def _build():
    from contextlib import ExitStack

    import concourse.tile as tile
    from concourse import bacc, mybir

    _patch_act_tables()

    f32 = mybir.dt.float32
    f16 = mybir.dt.float16
    bf16 = mybir.dt.bfloat16
    f8 = mybir.dt.float8e4
    Alu = mybir.AluOpType
    Act = mybir.ActivationFunctionType
    AX = mybir.AxisListType

    nc = bacc.Bacc("TRN2", target_bir_lowering=False, debug=False,
                   num_devices=NCORES)

    xh_ext = nc.dram_tensor("xh", [TLOC, C, N], bf16, kind="ExternalInput").ap()
    xl_ext = nc.dram_tensor("xl8", [TLOC, C, N], f8, kind="ExternalInput").ap()
    xt_ext = nc.dram_tensor("xt", [TLOC, N, C], f16, kind="ExternalInput").ap()
    wh_ext = nc.dram_tensor("wh", [128, CCH, K], bf16, kind="ExternalInput").ap()
    wl_ext = nc.dram_tensor("wl", [128, CCH, K], bf16, kind="ExternalInput").ap()
    whs_ext = nc.dram_tensor("whs", [128, CCH, K], f8, kind="ExternalInput").ap()
    ebb_ext = nc.dram_tensor("ebb", [128, K], f32, kind="ExternalInput").ap()
    cent2_ext = nc.dram_tensor("cent2", [128, C], f32, kind="ExternalInput").ap()
    id64_ext = nc.dram_tensor("id64", [K, K], f32, kind="ExternalInput").ap()
    sel2_ext = nc.dram_tensor("sel2", [128, 2], f32, kind="ExternalInput").ap()
    sel2t_ext = nc.dram_tensor("sel2t", [2, 128], f32, kind="ExternalInput").ap()
    maskm_ext = nc.dram_tensor("maskm", [128, 2], f32, kind="ExternalInput").ap()
    out_ext = nc.dram_tensor("out", [2, K, C], f32, kind="ExternalOutput").ap()

    with tile.TileContext(nc) as tc, ExitStack() as ctx:
        const = ctx.enter_context(tc.tile_pool(name="const", bufs=1))
        xp = ctx.enter_context(tc.tile_pool(name="xp", bufs=TLOC))
        xtp = ctx.enter_context(tc.tile_pool(name="xtp", bufs=TLOC))
        lg = ctx.enter_context(tc.tile_pool(name="lg", bufs=3))
        zp = ctx.enter_context(tc.tile_pool(name="zp", bufs=3))
        app = ctx.enter_context(tc.tile_pool(name="app", bufs=3))
        st = ctx.enter_context(tc.tile_pool(name="st", bufs=3))
        scr = ctx.enter_context(tc.tile_pool(name="scr", bufs=3))
        ep = ctx.enter_context(tc.tile_pool(name="ep", bufs=2))
        ps_lga = ctx.enter_context(tc.tile_pool(name="ps_lga", bufs=2, space="PSUM"))
        ps_lgb = ctx.enter_context(tc.tile_pool(name="ps_lgb", bufs=1, space="PSUM"))
        ps_lt = ctx.enter_context(tc.tile_pool(name="ps_lt", bufs=3, space="PSUM"))
        ps_vl = ctx.enter_context(tc.tile_pool(name="ps_vl", bufs=1, space="PSUM"))
        ps_ti = ctx.enter_context(tc.tile_pool(name="ps_ti", bufs=1, space="PSUM"))

        # load issue order = arrival order: t0's data, then the constants the
        # first timestep needs, then the remaining timesteps
        xh_t, xl_t, xt_t = [], [], []
        for t in range(TLOC):
            xh_sb = xp.tile([128, CCH, N], bf16, tag="xh")
            xt_sb = xtp.tile([NP, NCH, C], f16, tag="xt")
            xl_sb = xp.tile([128, CCH, N], f8, tag="xl")
            xh_t.append(xh_sb)
            xt_t.append(xt_sb)
            xl_t.append(xl_sb)

        def load_t(t):
            nc.sync.dma_start(
                xh_t[t][:], xh_ext[t].rearrange("(cc p) n -> p cc n", p=128))
            nc.sync.dma_start(
                xt_t[t][:], xt_ext[t].rearrange("(nn p) c -> p nn c", p=NP))
            nc.sync.dma_start(
                xl_t[t][:], xl_ext[t].rearrange("(cc p) n -> p cc n", p=128))

        nc.sync.dma_start(
            xh_t[0][:], xh_ext[0].rearrange("(cc p) n -> p cc n", p=128))
        wh_sb = const.tile([128, CCH, K], bf16, tag="wh")
        nc.sync.dma_start(wh_sb[:], wh_ext[:])
        wl_sb = const.tile([128, CCH, K], bf16, tag="wl")
        nc.sync.dma_start(wl_sb[:], wl_ext[:])
        whs_sb = const.tile([128, CCH, K], f8, tag="whs")
        nc.sync.dma_start(whs_sb[:], whs_ext[:])
        nc.sync.dma_start(
            xl_t[0][:], xl_ext[0].rearrange("(cc p) n -> p cc n", p=128))
        nc.sync.dma_start(
            xt_t[0][:], xt_ext[0].rearrange("(nn p) c -> p nn c", p=NP))
        id64_sb = const.tile([K, K], f32, tag="id64")
        nc.sync.dma_start(id64_sb[:], id64_ext[:])
        nc.sync.dma_start(
            xh_t[1][:], xh_ext[1].rearrange("(cc p) n -> p cc n", p=128))
        nc.sync.dma_start(
            xl_t[1][:], xl_ext[1].rearrange("(cc p) n -> p cc n", p=128))
        nc.sync.dma_start(
            xt_t[1][:], xt_ext[1].rearrange("(nn p) c -> p nn c", p=NP))
        ebb_sb = const.tile([128, K], f32, tag="ebb")
        nc.sync.dma_start(ebb_sb[:], ebb_ext[:])
        cent2_sb = const.tile([128, C], f32, tag="cent2")
        nc.sync.dma_start(cent2_sb[:], cent2_ext[:])
        sel2_sb = const.tile([128, 2], f32, tag="sel2")
        nc.sync.dma_start(sel2_sb[:], sel2_ext[:])
        sel2t_sb = const.tile([2, 128], f32, tag="sel2t")
        nc.sync.dma_start(sel2t_sb[:], sel2t_ext[:])
        maskm_sb = const.tile([128, 2], f32, tag="maskm")
        nc.sync.dma_start(maskm_sb[:], maskm_ext[:])
        nc.sync.dma_start(
            xh_t[2][:], xh_ext[2].rearrange("(cc p) n -> p cc n", p=128))
        nc.sync.dma_start(
            xl_t[2][:], xl_ext[2].rearrange("(cc p) n -> p cc n", p=128))
        nc.sync.dma_start(
            xt_t[2][:], xt_ext[2].rearrange("(nn p) c -> p nn c", p=NP))
        nc.sync.dma_start(
            xh_t[3][:], xh_ext[3].rearrange("(cc p) n -> p cc n", p=128))
        nc.sync.dma_start(
            xl_t[3][:], xl_ext[3].rearrange("(cc p) n -> p cc n", p=128))
        nc.sync.dma_start(
            xt_t[3][:], xt_ext[3].rearrange("(nn p) c -> p nn c", p=NP))
        acc = const.tile([128, C], f32, tag="acc")
        nc.vector.memset(acc[:], 0.0)

        def make_macro(mm_i):
            return {
                "i": mm_i,
                "pvl2": None, "pas2": None,
                "halves": [
                    {"t": 2 * mm_i + half, "r0": 64 * half,
                     "r1": 64 * half + 64, "xh": xh_t[2 * mm_i + half],
                     "xl": xl_t[2 * mm_i + half], "xt": xt_t[2 * mm_i + half]}
                    for half in range(2)
                ],
            }

        def stage_B(m):
            # logits matmuls, both timesteps back-to-back on the PE
            for h in m["halves"]:
                psla = ps_lga.tile([K, 512], f32, tag="psla")
                pslb = ps_lgb.tile([K, N - 512], f32, tag="pslb")
                passes = ((wh_sb, h["xh"]), (whs_sb, h["xl"]),
                          (wl_sb, h["xh"]))
                for ps_t, n0, n1 in ((psla, 0, 512), (pslb, 512, N)):
                    for pi, (wsb, xsb) in enumerate(passes):
                        for cc in range(CCH):
                            nc.tensor.matmul(
                                ps_t[:, 0:n1 - n0], wsb[:, cc, :],
                                xsb[:, cc, n0:n1],
                                start=(pi == 0 and cc == 0),
                                stop=(pi == 2 and cc == CCH - 1))
                h["psla"], h["pslb"] = psla, pslb

        def stage_A(m):
            # column norms from fp16 x^T
            for h in m["halves"]:
                ss = st.tile([NP, 8], f32, tag="ss")
                n_act = 4 if h["t"] == TLOC - 1 else 2
                for nn in range(NCH):
                    if nn < n_act:
                        s1 = scr.tile([NP, C], f16, tag="scr_a")
                        nc.scalar.activation(s1[:], h["xt"][:, nn, :],
                                             Act.Square,
                                             accum_out=ss[:, nn:nn + 1])
                    else:
                        s2 = scr.tile([NP, C], f16, tag="scr_d")
                        nc.vector.scalar_tensor_tensor(
                            s2[:], h["xt"][:, nn, :], 0.0, h["xt"][:, nn, :],
                            Alu.bypass, Alu.mult, accum_out=ss[:, nn:nn + 1])
                ssc = st.tile([NP, 8], f32, tag="ssc")
                nc.vector.tensor_scalar_max(ssc[:, 0:NCH], ss[:, 0:NCH], 1.0)
                lnv = st.tile([NP, 8], f32, tag="lnv")
                nc.scalar.activation(lnv[:, 0:NCH], ssc[:, 0:NCH], Act.Ln)
                y0 = st.tile([NP, 8], f32, tag="y0")
                nc.scalar.activation(y0[:, 0:NCH], lnv[:, 0:NCH], Act.Exp,
                                     scale=-0.5)
                ya = st.tile([NP, 8], f32, tag="ya")
                nc.vector.tensor_mul(ya[:, 0:NCH], y0[:, 0:NCH], y0[:, 0:NCH])
                yb = st.tile([NP, 8], f32, tag="yb")
                nc.vector.scalar_tensor_tensor(yb[:, 0:NCH], ssc[:, 0:NCH],
                                               -0.5, ya[:, 0:NCH],
                                               Alu.mult, Alu.mult)
                yc = st.tile([NP, 8], f32, tag="yc")
                nc.vector.tensor_scalar_add(yc[:, 0:NCH], yb[:, 0:NCH], 1.5)
                rnorm = st.tile([NP, 8], f32, tag="rnorm")
                nc.vector.tensor_mul(rnorm[:, 0:NCH], y0[:, 0:NCH],
                                     yc[:, 0:NCH])
                h["rnorm"] = rnorm

        def stage_C(m):
            # psum->sbuf logits copies + PE transposes
            for h in m["halves"]:
                lsb = lg.tile([K, N], f32, tag="lsb")
                nc.scalar.copy(lsb[:, 0:512], h["psla"][:])
                nc.scalar.copy(lsb[:, 512:N], h["pslb"][:])
                h["lsb"] = lsb
            for h in m["halves"]:
                plt = ps_lt.tile([NP, NCH, K], f32, tag="plt")
                for nn in range(NCH):
                    nc.tensor.transpose(plt[:, nn, :],
                                        h["lsb"][:, nn * NP:(nn + 1) * NP],
                                        id64_sb[:])
                h["plt"] = plt

        def stage_D(m):
            # softmax over k (conv bias as exp(b - bshift) factor)
            for h in m["halves"]:
                plt, rnorm = h["plt"], h["rnorm"]
                mx = st.tile([NP, 8], f32, tag="mx")
                nc.vector.reduce_max(mx[:, 0:NCH], plt[:, :, :], axis=AX.X)
                negmr = st.tile([NP, 8], f32, tag="negmr")
                nc.vector.scalar_tensor_tensor(negmr[:, 0:NCH], mx[:, 0:NCH],
                                               -1.0, rnorm[:, 0:NCH],
                                               Alu.mult, Alu.mult)
                e3 = zp.tile([NP, NCH, K], f32, tag="e3")
                for nn in range(NCH):
                    nc.scalar.activation(e3[:, nn, :], plt[:, nn, :], Act.Exp,
                                         bias=negmr[:, nn:nn + 1],
                                         scale=rnorm[:, nn:nn + 1])
                a3 = zp.tile([NP, NCH, K], f32, tag="a3")
                ssum = st.tile([NP, 8], f32, tag="ssum")
                for nn in range(NCH):
                    nc.vector.scalar_tensor_tensor(
                        a3[:, nn, :], e3[:, nn, :], 0.0, ebb_sb[0:NP, :],
                        Alu.bypass, Alu.mult, accum_out=ssum[:, nn:nn + 1])
                rs = st.tile([NP, 8], f32, tag="rs")
                nc.vector.reciprocal(rs[:, 0:NCH], ssum[:, 0:NCH])
                rs2 = st.tile([NP, 8], f32, tag="rs2")
                nc.vector.scalar_tensor_tensor(rs2[:, 0:NCH], rs[:, 0:NCH],
                                               SCL, rnorm[:, 0:NCH],
                                               Alu.mult, Alu.mult)
                apt = app.tile([NP, NCH, K], f16, tag="apt")
                for nn in range(NCH):
                    nc.vector.tensor_scalar_mul(apt[:, nn, :], a3[:, nn, :],
                                                rs2[:, nn:nn + 1])
                h["a3"], h["rs"], h["apt"] = a3, rs, apt

        def stage_E(m):
            # vlad aggregation
            pvl2 = ps_vl.tile([128, C], f32, tag="pvl2")
            pas2 = ps_ti.tile([128, 4], f32, tag="epi_ps")
            m["pvl2"], m["pas2"] = pvl2, pas2
            for h in m["halves"]:
                r0, r1 = h["r0"], h["r1"]
                for nn in range(NCH):
                    nc.tensor.matmul(m["pas2"][r0:r1, 0:1], h["a3"][:, nn, :],
                                     h["rs"][:, nn:nn + 1],
                                     start=(nn == 0), stop=(nn == NCH - 1))
                for nn in range(NCH):
                    nc.tensor.matmul(m["pvl2"][r0:r1, :], h["apt"][:, nn, :],
                                     h["xt"][:, nn, :],
                                     start=(nn == 0), stop=(nn == NCH - 1))

        def stage_epi(m):
            # epilogue for the two timesteps, packed [128, 512]
            pvl2, pas2, mm_i = m["pvl2"], m["pas2"], m["i"]
            vld = ep.tile([128, C], f32, tag="vld")
            nc.vector.scalar_tensor_tensor(vld[:], cent2_sb[:],
                                           pas2[:, 0:1], pvl2[:],
                                           Alu.mult, Alu.add)
            scr2 = scr.tile([128, C], bf16, tag="scr2")
            ss2 = ep.tile([128, 1], f32, tag="ss2")
            nc.scalar.activation(scr2[:], vld[:], Act.Square, accum_out=ss2[:])
            qc = ep.tile([128, 1], f32, tag="qc")
            nc.vector.tensor_scalar(qc[:], ss2[:], 1.0 / (C2 * C2), 1.0,
                                    Alu.mult, Alu.min)
            nc.tensor.matmul(pas2[0:2, 1:2], sel2_sb[:], qc[:],
                             start=True, stop=True)
            gsb = ep.tile([2, 1], f32, tag="gsb")
            nc.scalar.copy(gsb[:], pas2[0:2, 1:2])
            nc.tensor.matmul(pas2[:, 2:3], sel2t_sb[:], gsb[:],
                             start=True, stop=True)
            p2 = ep.tile([128, 2], f32, tag="p2")
            nc.vector.tensor_scalar_max(p2[:, 0:1], ss2[:], C2 * C2)
            nc.vector.tensor_scalar_max(p2[:, 1:2], pas2[:, 2:3], GEPS2)
            lp = ep.tile([128, 2], f32, tag="lp")
            nc.scalar.activation(lp[:], p2[:], Act.Ln)
            wr = ep.tile([128, 2], f32, tag="wr")
            nc.scalar.activation(wr[:], lp[:], Act.Exp, scale=-0.5)
            sk = ep.tile([128, 1], f32, tag="sk")
            nc.vector.scalar_tensor_tensor(
                sk[:], wr[:, 0:1], maskm_sb[:, mm_i:mm_i + 1], wr[:, 1:2],
                Alu.mult, Alu.mult)
            nc.vector.scalar_tensor_tensor(acc[:], vld[:], sk[:, 0:1], acc[:],
                                           Alu.mult, Alu.add)

        # hand-pipelined schedule: macro 2's matmul stages run while macro 1
        # is in its vector-bound softmax/epilogue stages
        m1 = make_macro(0)
        m2 = make_macro(1)
        stage_B(m1)
        stage_A(m1)
        stage_C(m1)
        stage_B(m2)
        stage_A(m2)
        stage_D(m1)
        stage_C(m2)
        stage_E(m1)
        stage_D(m2)
        stage_epi(m1)
        stage_E(m2)
        stage_epi(m2)

        nc.sync.dma_start(out_ext[0], acc[0:K, :])
        nc.sync.dma_start(out_ext[1], acc[K:128, :])

    nc.compile()
    return nc


def _get_nc():
    if "nc" not in _CACHE:
        _CACHE["nc"] = _build()
    return _CACHE["nc"]


def _in_maps(x1, centroids, conv_w, conv_b):
    import ml_dtypes

    bf16 = ml_dtypes.bfloat16
    x1 = np.asarray(x1, dtype=np.float32)
    centroids = np.asarray(centroids, dtype=np.float32)
    conv_w = np.asarray(conv_w, dtype=np.float32)
    conv_b = np.asarray(conv_b, dtype=np.float32)

    T = x1.shape[0]
    x = np.ascontiguousarray(x1.reshape(T, C, N))
    TP = NCORES * TLOC
    xpad = np.zeros((TP, C, N), dtype=np.float32)
    xpad[:T] = x
    f8 = ml_dtypes.float8_e4m3
    xh = xpad.astype(bf16)
    xl8 = ((xpad - xh.astype(np.float32)) * 128.0).astype(f8)

    wt = np.ascontiguousarray(conv_w.T.reshape(CCH, 128, K).transpose(1, 0, 2))
    wh = wt.astype(bf16)
    wl = (wt - wh.astype(np.float32)).astype(bf16)
    whs = (wt / 128.0).astype(f8)
    # softmax shift uses max_k(W@x)/norm + bshift: exp(b - bshift) must stay
    # within [denormal, e^40] (ssum <= 64*e^40 < 2^64, the ACT Ln limit), so
    # allow at most +40 above and ~85 below (valid for bias range < ~125)
    bmid = (conv_b.max() + conv_b.min()) / 2.0
    bshift = max(bmid, conv_b.max() - 40.0)
    eb = np.exp((conv_b - bshift).astype(np.float64)).astype(np.float32)
    ebb = np.ascontiguousarray(np.broadcast_to(eb[None, :], (128, K)))
    cent2 = np.ascontiguousarray(-SCL * np.vstack([centroids, centroids]))
    id64 = np.eye(K, dtype=np.float32)
    sel2 = np.zeros((128, 2), dtype=np.float32)
    sel2[0:64, 0] = 1.0
    sel2[64:128, 1] = 1.0
    sel2t = np.ascontiguousarray(sel2.T)

    in_maps = []
    for i in range(NCORES):
        sl = slice(i * TLOC, (i + 1) * TLOC)
        xti = np.ascontiguousarray(
            xpad[sl].transpose(0, 2, 1).astype(np.float16))
        maskm = np.zeros((128, 2), dtype=np.float32)
        for mi in range(2):
            for half in range(2):
                t = i * TLOC + 2 * mi + half
                if t < T:
                    maskm[64 * half:64 * half + 64, mi] = 1.0
        in_maps.append({
            "xh": np.ascontiguousarray(xh[sl]),
            "xl8": np.ascontiguousarray(xl8[sl]),
            "xt": xti, "wh": wh, "wl": wl, "whs": whs, "ebb": ebb,
            "cent2": cent2,
            "id64": id64, "sel2": sel2, "sel2t": sel2t, "maskm": maskm,
        })
    return in_maps


def kernel(x1, centroids, conv_w, conv_b):
    from concourse.bass_utils import run_bass_kernel_spmd

    in_maps = _in_maps(x1, centroids, conv_w, conv_b)
    nc = _get_nc()
    res = run_bass_kernel_spmd(nc, in_maps, core_ids=list(range(NCORES)))
    total = np.zeros((K, C), dtype=np.float32)
    for i in range(NCORES):
        total += res.results[i]["out"][0]
        total += res.results[i]["out"][1]
    return total.reshape(1, K * C)
